# revision 7
# baseline (speedup 1.0000x reference)
"""Trainium2 Bass kernel for nn_CodePredLayersExport_8650064134911.

5-layer dense transformer, batch=1 single-token decode. 8-way tensor
parallel: q/k/v/o sharded over heads, gate/up/down over intermediate dim,
one fp32 AllReduce after o_proj and one after down_proj per layer.

Weights are converted to bf16 on the host and laid out per-core in
LDWEIGHTS-ready column blocks ([contraction-chunk partition, out-tile free]).
The decode position is baked into the compiled program (host knows
position_id); RoPE tables and the rotate-half permutation are host-built.
KV-cache outputs are assembled on the host from device-computed k/v rows.
"""
import os
import sys

for p in ("/opt/trn_rl_repo", "/root/.axon_site", "/root/.axon_site/_ro/trn_rl_repo",
          "/root/.axon_site/_ro/pypackages"):
    if os.path.isdir(p) and p not in sys.path:
        sys.path.append(p)

import numpy as np
import ml_dtypes

import concourse.bass as bass
import concourse.bacc as bacc
import concourse.tile as tile
from concourse import mybir
from concourse.bass_utils import run_bass_kernel_spmd

BF16 = ml_dtypes.bfloat16

L, H, NH, NKV, HD, I, MAXS = 5, 1024, 16, 8, 128, 3072, 16
EPS = 1e-6
THETA = 1000000.0
NCORES = 8
QH = NH // NCORES            # 2 q heads per core
ISH = I // NCORES            # 384 intermediate per core
CH = H // 128                # 8 contraction chunks over hidden
OCH = QH * HD // 128         # 2 contraction chunks over attn dims
DCH = ISH // 128             # 3 contraction chunks over intermediate shard
QKT = QH + 1                 # 3 output tiles for fused q,q,k
GT = ISH // 128              # 3 output tiles for gate / up
F32 = mybir.dt.float32
BF = mybir.dt.bfloat16

_prog_cache = {}


def _build(pos):
    S = pos + 1
    nc = bacc.Bacc("TRN2", target_bir_lowering=False, debug=False,
                   enable_asserts=True, num_devices=NCORES)

    # ---- DRAM I/O ----
    d_wqk = nc.dram_tensor("wqk", [128, L * QKT * CH * 128], BF, kind="ExternalInput")
    d_wv = nc.dram_tensor("wv", [128, L * CH * 128], BF, kind="ExternalInput")
    d_wo = nc.dram_tensor("wo", [128, L * CH * OCH * 128], BF, kind="ExternalInput")
    d_wg = nc.dram_tensor("wg", [128, L * GT * CH * 128], BF, kind="ExternalInput")
    d_wu = nc.dram_tensor("wu", [128, L * GT * CH * 128], BF, kind="ExternalInput")
    d_wd = nc.dram_tensor("wd", [128, L * CH * DCH * 128], BF, kind="ExternalInput")
    d_kT = nc.dram_tensor("kT", [L, 128, MAXS], F32, kind="ExternalInput")
    d_vC = nc.dram_tensor("vC", [L, MAXS, 128], F32, kind="ExternalInput")
    d_x0 = nc.dram_tensor("x0", [128, CH], F32, kind="ExternalInput")
    d_rts = nc.dram_tensor("rts", [128, 128], F32, kind="ExternalInput")
    d_cos = nc.dram_tensor("cosc", [128, 1], F32, kind="ExternalInput")
    d_qkn = nc.dram_tensor("qkn", [L, 128, QKT], F32, kind="ExternalInput")
    d_onc = nc.dram_tensor("onc", [128, CH], F32, kind="ExternalInput")
    d_xout = nc.dram_tensor("x_out", [128, CH], F32, kind="ExternalOutput")
    d_kout = nc.dram_tensor("k_out", [L, 128], F32, kind="ExternalOutput")
    d_vout = nc.dram_tensor("v_out", [L, 128], F32, kind="ExternalOutput")

    AX = mybir.AxisListType.X
    AF = mybir.ActivationFunctionType

    with tile.TileContext(nc) as tc:
        with (
            tc.tile_pool(name="wpool", bufs=1) as wp,
            tc.tile_pool(name="small", bufs=1) as sp,
            tc.tile_pool(name="act", bufs=1) as ap,
            tc.tile_pool(name="ps_a", bufs=2, space="PSUM") as psb,
            tc.tile_pool(name="ps_b", bufs=2, space="PSUM") as psu,
            tc.tile_pool(name="ps_c", bufs=3, space="PSUM") as pss,
            tc.tile_pool(name="dram", bufs=1, space="DRAM") as dp,
        ):
            # ---- small inputs / constants ----
            x0 = sp.tile([128, CH], F32, tag="x0")
            nc.sync.dma_start(x0[:], d_x0.ap())
            rts = sp.tile([128, 128], F32, tag="rts")
            nc.sync.dma_start(rts[:], d_rts.ap())
            cosc = sp.tile([128, 1], F32, tag="cosc")
            nc.sync.dma_start(cosc[:], d_cos.ap())
            onc = sp.tile([128, CH], F32, tag="onc")
            nc.sync.dma_start(onc[:], d_onc.ap())
            qkn = []
            for l in range(L):
                t = sp.tile([128, QKT], F32, tag=f"qkn{l}")
                nc.sync.dma_start(t[:], d_qkn.ap()[l])
                qkn.append(t)
            kTs, vCs = [], []
            for l in range(L):
                t = sp.tile([128, MAXS], F32, tag=f"kT{l}")
                nc.sync.dma_start(t[:], d_kT.ap()[l])
                kTs.append(t)
                t = sp.tile([MAXS, 128], F32, tag=f"vC{l}")
                nc.sync.dma_start(t[:], d_vC.ap()[l])
                vCs.append(t)

            ones128 = sp.tile([128, 1], F32, tag="ones128")
            nc.vector.memset(ones128[:], 1.0)
            ones_r = sp.tile([1, 128], F32, tag="ones_r")
            nc.vector.memset(ones_r[:], 1.0)
            ones16 = sp.tile([MAXS, 1], F32, tag="ones16")
            nc.vector.memset(ones16[:], 1.0)
            epst = sp.tile([1, 1], F32, tag="epst")
            nc.vector.memset(epst[:], EPS)
            zero16 = sp.tile([MAXS, 1], F32, tag="zero16")
            nc.vector.memset(zero16[:], 0.0)
            zero128 = sp.tile([128, 1], F32, tag="zero128")
            nc.vector.memset(zero128[:], 0.0)

            # ---- weight tiles + DMAs (issued early, consumption order) ----
            wqk, wv, wo, wg, wu, wd = [], [], [], [], [], []
            for l in range(L):
                for lst, dt_, width in (
                    (wqk, d_wqk, QKT * CH * 128),
                    (wv, d_wv, CH * 128),
                    (wo, d_wo, CH * OCH * 128),
                    (wg, d_wg, GT * CH * 128),
                    (wu, d_wu, GT * CH * 128),
                    (wd, d_wd, CH * DCH * 128),
                ):
                    t = wp.tile([128, width], BF, tag=f"w{id(lst)}_{l}")
                    nc.sync.dma_start(t[:], dt_.ap()[:, l * width:(l + 1) * width])
                    lst.append(t)

            def rmsscalar(ltag, xt, n):
                """1/sqrt(mean over all n*... of x^2 + eps) -> [1,1] sbuf."""
                xsq = ap.tile([128, xt.shape[1]], F32, tag=f"xsq{ltag}")
                nc.vector.tensor_mul(xsq[:], xt[:], xt[:])
                cs = pss.tile([1, xt.shape[1]], F32, tag="psC")
                nc.tensor.matmul(cs[:], ones128[:], xsq[:], start=True, stop=True)
                ssum = ap.tile([1, 1], F32, tag=f"ss{ltag}")
                nc.vector.reduce_sum(ssum[:], cs[:], axis=AX)
                sq = ap.tile([1, 1], F32, tag=f"sq{ltag}")
                nc.scalar.activation(sq[:], ssum[:], AF.Sqrt, bias=epst[:],
                                     scale=1.0 / n)
                r = ap.tile([1, 1], F32, tag=f"r{ltag}")
                nc.vector.reciprocal(r[:], sq[:])
                return r

            x = x0
            for l in range(L):
                lt = f"l{l}"
                # --- r1 = rms scalar of x (only needed for the v path) ---
                r1 = rmsscalar(lt + "a", x, H)

                x_bf = ap.tile([128, CH], BF, tag=f"xbf{lt}")
                nc.vector.tensor_copy(x_bf[:], x[:])

                # --- q,q,k projections (weight-stationary, column outputs) ---
                qk_ps = psb.tile([128, QKT], F32, tag="psA")
                for t in range(QKT):
                    for c in range(CH):
                        off = (t * CH + c) * 128
                        nc.tensor.matmul(
                            qk_ps[:, t:t + 1],
                            wqk[l][:, off:off + 128],
                            x_bf[:, c:c + 1],
                            start=(c == 0), stop=(c == CH - 1))

                # --- v projection (x-stationary, row output) ---
                v_ps = psu.tile([1, 128], F32, tag="psB")
                for c in range(CH):
                    nc.tensor.matmul(
                        v_ps[:], x_bf[:, c:c + 1],
                        wv[l][:, c * 128:(c + 1) * 128],
                        start=(c == 0), stop=(c == CH - 1))
                v_sc = ap.tile([1, 128], F32, tag=f"vsc{lt}")
                nc.vector.tensor_scalar_mul(v_sc[:], v_ps[:], r1[:])
                # insert new v row into the cache at partition `pos` (DMA moves
                # across partitions; engines cannot)
                nc.sync.dma_start(vCs[l][pos:pos + 1, :], v_sc[:])
                nc.sync.dma_start(d_vout.ap()[l], v_sc[:])

                # --- q/k head RMS norm ---
                yqk = ap.tile([128, QKT], F32, tag=f"yqk{lt}")
                nc.vector.tensor_copy(yqk[:], qk_ps[:])
                ysq = ap.tile([128, QKT], F32, tag=f"ysq{lt}")
                nc.vector.tensor_mul(ysq[:], yqk[:], yqk[:])
                ncs = pss.tile([1, QKT], F32, tag="psC")
                nc.tensor.matmul(ncs[:], ones128[:], ysq[:], start=True, stop=True)
                nsq = ap.tile([1, QKT], F32, tag=f"nsq{lt}")
                nc.scalar.activation(nsq[:], ncs[:], AF.Sqrt, bias=epst[:],
                                     scale=1.0 / HD)
                nrec = ap.tile([1, QKT], F32, tag=f"nrec{lt}")
                nc.vector.reciprocal(nrec[:], nsq[:])
                nbc = pss.tile([128, QKT], F32, tag="psC")
                nc.tensor.matmul(nbc[:], ones_r[:], nrec[:], start=True, stop=True)
                yn = ap.tile([128, QKT], F32, tag=f"yn{lt}")
                nc.vector.tensor_mul(yn[:], yqk[:], nbc[:])
                nc.vector.tensor_mul(yn[:], yn[:], qkn[l][:])

                # --- RoPE: out = yn*cos + (diag(sin) @ Rot) @ yn ---
                rot_ps = pss.tile([128, QKT], F32, tag="psC")
                nc.tensor.matmul(rot_ps[:], rts[:], yn[:], start=True, stop=True)
                qkr = ap.tile([128, QKT], F32, tag=f"qkr{lt}")
                nc.vector.tensor_scalar_mul(qkr[:], yn[:], cosc[:])
                nc.vector.tensor_add(qkr[:], qkr[:], rot_ps[:])

                # --- insert new k column, emit k/v cache updates ---
                nc.vector.tensor_copy(kTs[l][:, pos:pos + 1], qkr[:, QH:QH + 1])
                nc.sync.dma_start(d_kout.ap()[l], qkr[:, QH:QH + 1])

                # --- attention: scoresT [S, QH] = kT.T @ q ---
                scT = pss.tile([MAXS, QH], F32, tag="psC")
                nc.tensor.matmul(scT[:S, :], kTs[l][:, :S], qkr[:, 0:QH],
                                 start=True, stop=True)
                expT = ap.tile([MAXS, QH], F32, tag=f"expT{lt}")
                nc.scalar.activation(expT[:S, :], scT[:S, :], AF.Exp,
                                     bias=zero16[:S, :],
                                     scale=1.0 / float(np.sqrt(HD)))
                sums = pss.tile([1, QH], F32, tag="psC")
                nc.tensor.matmul(sums[:], ones16[:S, :], expT[:S, :],
                                 start=True, stop=True)
                srec = ap.tile([1, QH], F32, tag=f"srec{lt}")
                nc.vector.reciprocal(srec[:], sums[:])
                pbc = pss.tile([MAXS, QH], F32, tag="psC")
                nc.tensor.matmul(pbc[:S, :], ones_r[:, :S], srec[:],
                                 start=True, stop=True)
                pT = ap.tile([MAXS, QH], F32, tag=f"pT{lt}")
                nc.vector.tensor_mul(pT[:S, :], expT[:S, :], pbc[:S, :])
                attn_ps = psu.tile([128, QH], F32, tag="psB")
                nc.tensor.matmul(attn_ps[:], vCs[l][:S, :], pT[:S, :],
                                 start=True, stop=True)
                attn_bf = ap.tile([128, QH], BF, tag=f"attnbf{lt}")
                nc.vector.tensor_copy(attn_bf[:], attn_ps[:])

                # --- o projection partial [128, 8] ---
                o_ps = psb.tile([128, CH], F32, tag="psA")
                for t in range(CH):
                    for c in range(OCH):
                        off = (t * OCH + c) * 128
                        nc.tensor.matmul(
                            o_ps[:, t:t + 1],
                            wo[l][:, off:off + 128],
                            attn_bf[:, c:c + 1],
                            start=(c == 0), stop=(c == OCH - 1))
                o_sb = ap.tile([128, CH], F32, tag=f"osb{lt}")
                nc.vector.tensor_copy(o_sb[:], o_ps[:])

                # --- AllReduce #1 ---
                ar_in = dp.tile([128, CH], F32, tag=f"ari1{lt}")
                ar_out = dp.tile([128, CH], F32, tag=f"aro1{lt}")
                nc.sync.dma_start(ar_in[:], o_sb[:])
                nc.gpsimd.collective_compute(
                    "AllReduce", mybir.AluOpType.add,
                    ins=[ar_in.opt()], outs=[ar_out.opt()],
                    replica_groups=[list(range(NCORES))])
                delta = ap.tile([128, CH], F32, tag=f"dl1{lt}")
                nc.sync.dma_start(delta[:], ar_out[:])
                x2 = ap.tile([128, CH], F32, tag=f"x2{lt}")
                nc.vector.tensor_add(x2[:], x[:], delta[:])

                # --- FFN ---
                r2 = rmsscalar(lt + "b", x2, H)
                r2bc_ps = pss.tile([128, 1], F32, tag="psC")
                nc.tensor.matmul(r2bc_ps[:], ones_r[:], r2[:], start=True, stop=True)
                r2bc = ap.tile([128, 1], F32, tag=f"r2bc{lt}")
                nc.vector.tensor_copy(r2bc[:], r2bc_ps[:])
                x2_bf = ap.tile([128, CH], BF, tag=f"x2bf{lt}")
                nc.vector.tensor_copy(x2_bf[:], x2[:])

                g_ps = psb.tile([128, GT], F32, tag="psA")
                u_ps = psu.tile([128, GT], F32, tag="psB")
                for dst, w in ((g_ps, wg[l]), (u_ps, wu[l])):
                    for t in range(GT):
                        for c in range(CH):
                            off = (t * CH + c) * 128
                            nc.tensor.matmul(
                                dst[:, t:t + 1],
                                w[:, off:off + 128],
                                x2_bf[:, c:c + 1],
                                start=(c == 0), stop=(c == CH - 1))
                g_s = ap.tile([128, GT], F32, tag=f"gs{lt}")
                nc.vector.tensor_scalar_mul(g_s[:], g_ps[:], r2bc[:])
                u_s = ap.tile([128, GT], F32, tag=f"us{lt}")
                nc.vector.tensor_scalar_mul(u_s[:], u_ps[:], r2bc[:])
                g_a = ap.tile([128, GT], F32, tag=f"ga{lt}")
                nc.scalar.activation(g_a[:], g_s[:], AF.Silu, bias=zero128[:])
                ff = ap.tile([128, GT], BF, tag=f"ff{lt}")
                nc.vector.tensor_mul(ff[:], g_a[:], u_s[:])

                d_ps = psb.tile([128, CH], F32, tag="psA")
                for t in range(CH):
                    for c in range(DCH):
                        off = (t * DCH + c) * 128
                        nc.tensor.matmul(
                            d_ps[:, t:t + 1],
                            wd[l][:, off:off + 128],
                            ff[:, c:c + 1],
                            start=(c == 0), stop=(c == DCH - 1))
                dn_sb = ap.tile([128, CH], F32, tag=f"dnsb{lt}")
                nc.vector.tensor_copy(dn_sb[:], d_ps[:])

                # --- AllReduce #2 ---
                ar_in2 = dp.tile([128, CH], F32, tag=f"ari2{lt}")
                ar_out2 = dp.tile([128, CH], F32, tag=f"aro2{lt}")
                nc.sync.dma_start(ar_in2[:], dn_sb[:])
                nc.gpsimd.collective_compute(
                    "AllReduce", mybir.AluOpType.add,
                    ins=[ar_in2.opt()], outs=[ar_out2.opt()],
                    replica_groups=[list(range(NCORES))])
                delta2 = ap.tile([128, CH], F32, tag=f"dl2{lt}")
                nc.sync.dma_start(delta2[:], ar_out2[:])
                x3 = ap.tile([128, CH], F32, tag=f"x3{lt}")
                nc.vector.tensor_add(x3[:], x2[:], delta2[:])
                x = x3

            # --- final norm ---
            r3 = rmsscalar("fin", x, H)
            r3bc_ps = pss.tile([128, 1], F32, tag="psC")
            nc.tensor.matmul(r3bc_ps[:], ones_r[:], r3[:], start=True, stop=True)
            r3bc = ap.tile([128, 1], F32, tag="r3bc_sb")
            nc.vector.tensor_copy(r3bc[:], r3bc_ps[:])
            xo = ap.tile([128, CH], F32, tag="xo")
            nc.vector.tensor_scalar_mul(xo[:], x[:], r3bc[:])
            nc.vector.tensor_mul(xo[:], xo[:], onc[:])
            nc.sync.dma_start(d_xout.ap(), xo[:])

    nc.compile()
    return nc


def _prep_core_inputs(inputs, pos):
    """Per-core host-side weight shuffles -> list of in_maps."""
    wq = np.asarray(inputs["wq"], np.float32)
    wk = np.asarray(inputs["wk"], np.float32)
    wv = np.asarray(inputs["wv"], np.float32)
    wo = np.asarray(inputs["wo"], np.float32)
    wg = np.asarray(inputs["w_gate"], np.float32)
    wu = np.asarray(inputs["w_up"], np.float32)
    wd = np.asarray(inputs["w_down"], np.float32)
    w_iln = np.asarray(inputs["w_iln"], np.float32)
    w_paln = np.asarray(inputs["w_paln"], np.float32)
    w_qn = np.asarray(inputs["w_qn"], np.float32)
    w_kn = np.asarray(inputs["w_kn"], np.float32)
    pk = np.asarray(inputs["past_keys"], np.float32)
    pv = np.asarray(inputs["past_values"], np.float32)
    hs = np.asarray(inputs["hidden_states"], np.float32)
    onorm = np.asarray(inputs["w_onorm"], np.float32)

    # RoPE tables at pos
    inv_freq = 1.0 / (THETA ** (np.arange(0, HD, 2, dtype=np.float32) / HD))
    freqs = np.float32(pos) * inv_freq
    cos = np.concatenate([np.cos(freqs)] * 2).astype(np.float32)   # [128]
    sin = np.concatenate([np.sin(freqs)] * 2).astype(np.float32)
    # rotate-half permutation with signs, sin folded in:
    # rot(v)[d] = -v[d+64] (d<64) ; v[d-64] (d>=64)
    R = np.zeros((HD, HD), np.float32)
    for d in range(64):
        R[d, d + 64] = -1.0
        R[d + 64, d] = 1.0
    rts = (np.diag(sin) @ R).T.astype(np.float32)                  # lhsT
    cosc = cos.reshape(HD, 1)

    def wstat(Wp):      # [rows(out), 1024] -> [128, rows/128 * 8 * 128]
        T = Wp.shape[0] // 128
        return np.ascontiguousarray(
            Wp.reshape(T, 128, CH, 128).transpose(3, 0, 2, 1).reshape(128, -1))

    def wstat_d(Dp):    # [1024(out), 384(in)] -> [128, 8*3*128]
        return np.ascontiguousarray(
            Dp.reshape(CH, 128, DCH, 128).transpose(3, 0, 2, 1).reshape(128, -1))

    in_maps = []
    for i in range(NCORES):
        m = {}
        wqk_l, wv_l, wo_l, wg_l, wu_l, wd_l = [], [], [], [], [], []
        for l in range(L):
            iln = w_iln[l][None, :]
            paln = w_paln[l][None, :]
            qs = wq[l][i * QH * HD:(i + 1) * QH * HD] * iln      # [256,1024]
            ks = wk[l][i * HD:(i + 1) * HD] * iln                # [128,1024]
            vs = wv[l][i * HD:(i + 1) * HD] * iln                # [128,1024]
            wqk_l.append(wstat(np.concatenate([qs, ks], 0)))
            # v is x-stationary: rhs chunk [128(k),128(n)] = Wv'[n, c*128+k]
            wv_l.append(np.ascontiguousarray(
                vs.reshape(128, CH, 128).transpose(2, 1, 0).reshape(128, -1)))
            os_ = wo[l][:, i * QH * HD:(i + 1) * QH * HD]        # [1024,256]
            wo_l.append(np.ascontiguousarray(
                os_.reshape(CH, 128, OCH, 128).transpose(3, 0, 2, 1).reshape(128, -1)))
            gs = wg[l][i * ISH:(i + 1) * ISH] * paln             # [384,1024]
            us = wu[l][i * ISH:(i + 1) * ISH] * paln
            wg_l.append(wstat(gs))
            wu_l.append(wstat(us))
            ds = wd[l][:, i * ISH:(i + 1) * ISH]                 # [1024,384]
            wd_l.append(wstat_d(ds))
        m["wqk"] = np.concatenate(wqk_l, 1).astype(BF16)
        m["wv"] = np.concatenate(wv_l, 1).astype(BF16)
        m["wo"] = np.concatenate(wo_l, 1).astype(BF16)
        m["wg"] = np.concatenate(wg_l, 1).astype(BF16)
        m["wu"] = np.concatenate(wu_l, 1).astype(BF16)
        m["wd"] = np.concatenate(wd_l, 1).astype(BF16)
        m["kT"] = np.ascontiguousarray(pk[:, 0, i].transpose(0, 2, 1))  # [L,128,16]
        m["vC"] = np.ascontiguousarray(pv[:, 0, i])                     # [L,16,128]
        m["x0"] = np.ascontiguousarray(hs[0, 0].reshape(CH, 128).T)
        m["rts"] = rts
        m["cosc"] = cosc
        qkn = np.stack([np.stack([w_qn[l], w_qn[l], w_kn[l]], 1) for l in range(L)])
        m["qkn"] = np.ascontiguousarray(qkn)                            # [L,128,3]
        m["onc"] = np.ascontiguousarray(onorm.reshape(CH, 128).T)
        in_maps.append(m)
    return in_maps


def kernel(**inputs):
    pos = int(np.asarray(inputs["position_id"]).reshape(-1)[0])
    if pos not in _prog_cache:
        _prog_cache[pos] = _build(pos)
    nc = _prog_cache[pos]

    in_maps = _prep_core_inputs(inputs, pos)
    res = run_bass_kernel_spmd(nc, in_maps, core_ids=list(range(NCORES)))

    out = res.results[0]["x_out"].T.reshape(1, 1, H).astype(np.float32)
    pks = np.array(np.asarray(inputs["past_keys"], np.float32))
    pvs = np.array(np.asarray(inputs["past_values"], np.float32))
    for i in range(NCORES):
        pks[:, 0, i, pos, :] = res.results[i]["k_out"]
        pvs[:, 0, i, pos, :] = res.results[i]["v_out"]
    return out, pks, pvs


if __name__ == "__main__":
    rng = np.random.default_rng(0)
    ins = {
        "hidden_states": rng.standard_normal((1, 1, H), dtype=np.float32),
        "position_id": np.array([5], np.int32),
        "past_keys": rng.standard_normal((L, 1, NKV, MAXS, HD), dtype=np.float32),
        "past_values": rng.standard_normal((L, 1, NKV, MAXS, HD), dtype=np.float32),
        "w_iln": np.ones((L, H), np.float32),
        "w_paln": np.ones((L, H), np.float32),
        "wq": (rng.standard_normal((L, NH * HD, H), dtype=np.float32) * 0.02),
        "wk": (rng.standard_normal((L, NKV * HD, H), dtype=np.float32) * 0.02),
        "wv": (rng.standard_normal((L, NKV * HD, H), dtype=np.float32) * 0.02),
        "wo": (rng.standard_normal((L, H, NH * HD), dtype=np.float32) * 0.02),
        "w_qn": np.ones((L, HD), np.float32),
        "w_kn": np.ones((L, HD), np.float32),
        "w_gate": (rng.standard_normal((L, I, H), dtype=np.float32) * 0.02),
        "w_up": (rng.standard_normal((L, I, H), dtype=np.float32) * 0.02),
        "w_down": (rng.standard_normal((L, H, I), dtype=np.float32) * 0.02),
        "w_onorm": np.ones((H,), np.float32),
    }
    o, k_, v_ = kernel(**ins)
    print("out", o.shape, o[0, 0, :4])
